# revision 1
# baseline (speedup 1.0000x reference)
"""LoRA linear layer on 8 Trainium2 NeuronCores.

Computes y = x @ W^T + b + 2.0 * (x @ A^T) @ B^T for
x:[4,4096,1024], W:[1024,1024], b:[1024], A:[16,1024], B:[1024,16].

Host side folds the LoRA update into the weight (W_eff = W + 2*B@A, an exact
algebraic identity), so the device kernel is a single GEMM + bias. Sharding is
data-parallel over the 16384 tokens: each of the 8 cores computes a
[2048, 1024] output slice with replicated weights.

Device kernel (per core): y_c[m,o] = sum_d xT_c[d,m] * WeffT[d,o] + b[o]
  - xT_c  [1024, 2048] f32 (host-transposed so the contraction dim d lands on
    SBUF partitions for both matmul operands)
  - WeffT [1024, 1024] f32, fully resident in SBUF
  - float32r matmuls (full PE rate at N=512), fp32 PSUM accumulation
  - bias broadcast to 128 partitions on host; fused add on the DVE during
    PSUM->SBUF eviction
"""

import numpy as np

import concourse.mybir as mybir
import concourse.tile as tile
from concourse import bacc
from concourse.bass_utils import run_bass_kernel_spmd

N_CORES = 8
P = 128
D = 1024  # in_features (contraction)
O = 1024  # out_features
M_TOTAL = 4 * 4096  # tokens
M = M_TOTAL // N_CORES  # tokens per core
KO = D // P  # k-subtiles
SC = 512  # m super-chunk (DMA granularity)
SCALING = 2.0

# Set by test harnesses to capture profiling info; harmless otherwise.
TRACE = False
LAST_RESULT = None

_NC_CACHE = None


def _build_nc():
    f32 = mybir.dt.float32
    f32r = mybir.dt.float32r

    nc = bacc.Bacc("TRN2", debug=False)
    xT = nc.dram_tensor("xT", [D, M], f32r, kind="ExternalInput")
    wT = nc.dram_tensor("wT", [D, O], f32r, kind="ExternalInput")
    bias = nc.dram_tensor("bias", [P, O], f32, kind="ExternalInput")
    y = nc.dram_tensor("y", [M, O], f32, kind="ExternalOutput")

    xT_v = xT[:].rearrange("(ko p) m -> p ko m", p=P)  # [128, 8, 2048]
    wT_v = wT[:].rearrange("(ko p) o -> p ko o", p=P)  # [128, 8, 1024]
    y_v = y[:].rearrange("(mt p) o -> p mt o", p=P)  # [128, 16, 1024]

    n_sc = M // SC
    with tile.TileContext(nc) as tc:
        with (
            tc.tile_pool(name="wpool", bufs=1) as wpool,
            tc.tile_pool(name="bpool", bufs=1) as bpool,
            tc.tile_pool(name="xpool", bufs=16) as xpool,
            tc.tile_pool(name="opool", bufs=6) as opool,
            tc.tile_pool(name="psum", bufs=8, space="PSUM") as psum,
        ):
            # x arrives in 256 KiB per-(super-chunk, ko) granules and W in
            # 512 KiB per-ko granules, so every matmul group only ever waits on
            # a small slice and DMA catch-up shows up as many short PE gaps
            # (which keep the HAM clock-gate warm) instead of multi-us stalls.
            xts = {}

            def load_x(sc):
                for ko in range(KO):
                    t = xpool.tile([P, SC], f32r, tag="xt", name=f"x{sc}_{ko}")
                    nc.sync.dma_start(t[:], xT_v[:, ko, sc * SC : (sc + 1) * SC])
                    xts[(sc, ko)] = t

            wt = [None] * KO

            def load_w(ko):
                t = wpool.tile([P, O], f32r, tag=f"w{ko}")
                nc.sync.dma_start(t[:], wT_v[:, ko, :])
                wt[ko] = t

            # Zero warmup tile: ~14 throwaway matmuls keep the PE busy while
            # the first x/W slices stream in, so the HAM clock-gate is warm
            # (2.4 GHz) by the time real matmuls start.
            zt = wpool.tile([P, 512], mybir.dt.bfloat16, tag="warm")
            nc.gpsimd.memset(zt[:], 0.0)
            wps = psum.tile([P, 512], mybir.dt.float32, tag="ps", name="wps")
            for _ in range(12):
                nc.tensor.matmul(wps[:], zt[:, :P], zt[:], start=True, stop=True)

            # sc0's x interleaved with W slices so the first real matmul only
            # waits on ~0.75 MiB; bias right behind (it gates all evictions).
            for ko in range(KO):
                t = xpool.tile([P, SC], f32r, tag="xt", name=f"x0_{ko}")
                nc.sync.dma_start(t[:], xT_v[:, ko, 0:SC])
                xts[(0, ko)] = t
                load_w(ko)
            bt = bpool.tile([P, O], f32)
            nc.sync.dma_start(bt[:], bias[:])

            def x_slice(sc, ko, mt_i):
                return xts[(sc, ko)][:, mt_i * P : (mt_i + 1) * P]

            def evict_half(ps, ot, half):
                nc.vector.tensor_tensor(
                    ot[:, half * 512 : (half + 1) * 512],
                    ps[:],
                    bt[:, half * 512 : (half + 1) * 512],
                    mybir.AluOpType.add,
                )

            MPC = SC // P  # m-tiles per super-chunk

            # Every super-chunk runs ko-outer: all four m-tiles accumulate
            # simultaneously across the 8 single-bank PSUM groups, so each W/x
            # slice is consumed as it lands during the ramp and the PE never
            # sits behind one large dependency. Evictions + stores are inlined
            # right behind each group's stop so PSUM slots recycle smoothly
            # into the next super-chunk.
            for sc in range(n_sc - 1):
                if sc + 1 < n_sc:
                    load_x(sc + 1)
                pss = [
                    [
                        psum.tile(
                            [P, 512], mybir.dt.float32, tag="ps", name=f"ps{sc}_{i}_{h}"
                        )
                        for h in range(2)
                    ]
                    for i in range(MPC)
                ]
                ots = [
                    opool.tile([P, O], f32, tag="ot", name=f"ot{sc}_{i}")
                    for i in range(MPC)
                ]
                for ko in range(KO):
                    last = ko == KO - 1
                    for mt_i in range(MPC):
                        mt = sc * MPC + mt_i
                        for half in range(2):
                            nc.tensor.matmul(
                                pss[mt_i][half][:],
                                x_slice(sc, ko, mt_i),
                                wt[ko][:, half * 512 : (half + 1) * 512],
                                start=ko == 0,
                                stop=last,
                            )
                        if last:
                            for half in range(2):
                                evict_half(pss[mt_i][half], ots[mt_i], half)
                                nc.gpsimd.dma_start(
                                    y_v[:, mt, half * 512 : (half + 1) * 512],
                                    ots[mt_i][:, half * 512 : (half + 1) * 512],
                                )

            # Last super-chunk: mt-outer, so evictions and stores spread across
            # its whole span instead of piling up after the final matmul; the
            # very last m-tile runs its two output halves back to back so
            # half 0's eviction/store hides under half 1's matmuls.
            sc = n_sc - 1
            for mt_i in range(MPC):
                mt = sc * MPC + mt_i
                ot = opool.tile([P, O], f32, tag="ot", name=f"otf{mt_i}")
                final = mt_i == MPC - 1
                if not final:
                    ph = [
                        psum.tile([P, 512], mybir.dt.float32, tag="ps", name=f"pl{h}")
                        for h in range(2)
                    ]
                    for ko in range(KO):
                        for half in range(2):
                            nc.tensor.matmul(
                                ph[half][:],
                                x_slice(sc, ko, mt_i),
                                wt[ko][:, half * 512 : (half + 1) * 512],
                                start=ko == 0,
                                stop=ko == KO - 1,
                            )
                    for half in range(2):
                        evict_half(ph[half], ot, half)
                        nc.gpsimd.dma_start(
                            y_v[:, mt, half * 512 : (half + 1) * 512],
                            ot[:, half * 512 : (half + 1) * 512],
                        )
                else:
                    for half in range(2):
                        ps = psum.tile([P, 512], mybir.dt.float32, tag="ps", name="pf")
                        for ko in range(KO):
                            nc.tensor.matmul(
                                ps[:],
                                x_slice(sc, ko, mt_i),
                                wt[ko][:, half * 512 : (half + 1) * 512],
                                start=ko == 0,
                                stop=ko == KO - 1,
                            )
                        evict_half(ps, ot, half)
                        nc.sync.dma_start(
                            y_v[:, mt, half * 512 : (half + 1) * 512],
                            ot[:, half * 512 : (half + 1) * 512],
                        )

    nc.compile()
    return nc


def _get_nc():
    global _NC_CACHE
    if _NC_CACHE is None:
        _NC_CACHE = _build_nc()
    return _NC_CACHE


def kernel(x, W, b, A, B):
    global LAST_RESULT
    x = np.ascontiguousarray(np.asarray(x, dtype=np.float32))
    W = np.asarray(W, dtype=np.float32)
    b = np.asarray(b, dtype=np.float32)
    A = np.asarray(A, dtype=np.float32)
    B = np.asarray(B, dtype=np.float32)
    assert x.shape == (4, 4096, D) and W.shape == (O, D)
    assert b.shape == (O,) and A.shape[1] == D and B.shape[0] == O

    # Fold the LoRA update into the weight: x@W^T + s*(x@A^T)@B^T = x@(W + s*B@A)^T
    Weff = (
        W.astype(np.float64) + SCALING * (B.astype(np.float64) @ A.astype(np.float64))
    ).astype(np.float32)
    WeffT = np.ascontiguousarray(Weff.T)  # [D, O]
    bias_rep = np.ascontiguousarray(np.broadcast_to(b[None, :], (P, O)))

    xr = x.reshape(M_TOTAL, D)
    in_maps = []
    for c in range(N_CORES):
        xTc = np.ascontiguousarray(xr[c * M : (c + 1) * M].T)  # [D, M]
        in_maps.append({"xT": xTc, "wT": WeffT, "bias": bias_rep})

    nc = _get_nc()
    res = run_bass_kernel_spmd(
        nc, in_maps, core_ids=list(range(N_CORES)), trace=TRACE
    )
    LAST_RESULT = res

    out = np.concatenate([res.results[c]["y"] for c in range(N_CORES)], axis=0)
    return out.reshape(x.shape[0], x.shape[1], O)



# revision 2
# speedup vs baseline: 1.0539x; 1.0539x over previous
"""LoRA linear layer on 8 Trainium2 NeuronCores.

Computes y = x @ W^T + b + 2.0 * (x @ A^T) @ B^T for
x:[4,4096,1024], W:[1024,1024], b:[1024], A:[16,1024], B:[1024,16].

Host side folds the LoRA update into the weight (W_eff = W + 2*B@A, an exact
algebraic identity), so the device kernel is a single GEMM + bias. Sharding is
data-parallel over the 16384 tokens: each of the 8 cores computes a
[2048, 1024] output slice with replicated weights.

Device kernel (per core): y_c[m,o] = sum_d xT_c[d,m] * WeffT[d,o] + b[o]
  - xT_c  [1024, 2048] bf16 (host-transposed + cast; contraction dim d on
    SBUF partitions for both matmul operands)
  - WeffT [1024, 1024] bf16, fully resident in SBUF as 16 [128,512] half-tiles
  - bf16 matmuls at N=512 (1 col/cycle, FWL weight loads), fp32 PSUM
  - bias (fp32, broadcast to 128 partitions on host) fused into the DVE
    PSUM->SBUF eviction, output cast to bf16 there; host upcasts to fp32
"""

import numpy as np
import ml_dtypes

import concourse.mybir as mybir
import concourse.tile as tile
from concourse import bacc
from concourse.bass_utils import run_bass_kernel_spmd

N_CORES = 8
P = 128
D = 1024  # in_features (contraction)
O = 1024  # out_features
M_TOTAL = 4 * 4096  # tokens
M = M_TOTAL // N_CORES  # tokens per core
KO = D // P  # k-subtiles
SC = 512  # m super-chunk (DMA granularity)
SCALING = 2.0
N_WARM = 12  # cold-rate N=128 warmup matmuls bridging the first DMA latency

# Set by test harnesses to capture profiling info; harmless otherwise.
TRACE = False
LAST_RESULT = None

_NC_CACHE = None


def _build_nc():
    f32 = mybir.dt.float32
    bf16 = mybir.dt.bfloat16

    nc = bacc.Bacc("TRN2", debug=False)
    xT = nc.dram_tensor("xT", [D, M], bf16, kind="ExternalInput")
    wT = nc.dram_tensor("wT", [D, O], bf16, kind="ExternalInput")
    bias = nc.dram_tensor("bias", [P, O], f32, kind="ExternalInput")
    y = nc.dram_tensor("y", [M, O], bf16, kind="ExternalOutput")

    xT_v = xT[:].rearrange("(ko p) m -> p ko m", p=P)  # [128, 8, 2048]
    wT_v = wT[:].rearrange("(ko p) o -> p ko o", p=P)  # [128, 8, 1024]
    y_v = y[:].rearrange("(mt p) o -> p mt o", p=P)  # [128, 16, 1024]

    n_sc = M // SC
    with tile.TileContext(nc) as tc:
        with (
            tc.tile_pool(name="wpool", bufs=1) as wpool,
            tc.tile_pool(name="bpool", bufs=1) as bpool,
            tc.tile_pool(name="xpool", bufs=16) as xpool,
            tc.tile_pool(name="opool", bufs=6) as opool,
            tc.tile_pool(name="psum", bufs=8, space="PSUM") as psum,
        ):
            xts = {}

            def load_x(sc):
                for ko in range(KO):
                    t = xpool.tile([P, SC], bf16, tag="xt", name=f"x{sc}_{ko}")
                    nc.sync.dma_start(t[:], xT_v[:, ko, sc * SC : (sc + 1) * SC])
                    xts[(sc, ko)] = t

            # W half-tiles [128, 512]: the first matmul group gates on 128 KiB
            # instead of 256 KiB.
            wt = [[None, None] for _ in range(KO)]

            def load_w(ko):
                for h in range(2):
                    t = wpool.tile([P, 512], bf16, tag=f"w{ko}_{h}")
                    nc.sync.dma_start(t[:], wT_v[:, ko, h * 512 : (h + 1) * 512])
                    wt[ko][h] = t

            # Warmup: short N=128 matmuls on a zeroed tile keep the PE busy
            # from ~body start so the HAM clock-gate window (3.4 us of
            # sustained activity) elapses while the first x/W slices stream
            # in; real matmuls then take over and finish the warm-up.
            zt = wpool.tile([P, P], bf16, tag="warm")
            nc.gpsimd.memset(zt[:], 0.0)
            wps = psum.tile([P, 512], mybir.dt.float32, tag="ps", name="wps")
            for _ in range(N_WARM):
                nc.tensor.matmul(wps[:, :P], zt[:], zt[:], start=True, stop=True)

            # sc0's x interleaved with W slices so the first real matmul only
            # waits on ~0.25 MiB; bias right behind (it gates all evictions).
            for ko in range(KO):
                t = xpool.tile([P, SC], bf16, tag="xt", name=f"x0_{ko}")
                nc.sync.dma_start(t[:], xT_v[:, ko, 0:SC])
                xts[(0, ko)] = t
                load_w(ko)
            bt = bpool.tile([P, O], f32)
            nc.sync.dma_start(bt[:], bias[:])

            def x_slice(sc, ko, mt_i):
                return xts[(sc, ko)][:, mt_i * P : (mt_i + 1) * P]

            def evict_half(ps, ot, half):
                nc.vector.tensor_tensor(
                    ot[:, half * 512 : (half + 1) * 512],
                    ps[:],
                    bt[:, half * 512 : (half + 1) * 512],
                    mybir.AluOpType.add,
                )

            MPC = SC // P  # m-tiles per super-chunk

            # Every super-chunk runs ko-outer: all four m-tiles accumulate
            # simultaneously across the 8 single-bank PSUM groups, so each W/x
            # slice is consumed as it lands during the ramp and the PE never
            # sits behind one large dependency. Evictions + stores are inlined
            # right behind each group's stop so PSUM slots recycle smoothly
            # into the next super-chunk.
            for sc in range(n_sc - 1):
                if sc + 1 < n_sc:
                    load_x(sc + 1)
                pss = [
                    [
                        psum.tile(
                            [P, 512], mybir.dt.float32, tag="ps", name=f"ps{sc}_{i}_{h}"
                        )
                        for h in range(2)
                    ]
                    for i in range(MPC)
                ]
                ots = [
                    opool.tile([P, O], bf16, tag="ot", name=f"ot{sc}_{i}")
                    for i in range(MPC)
                ]
                for ko in range(KO):
                    last = ko == KO - 1
                    for mt_i in range(MPC):
                        mt = sc * MPC + mt_i
                        for half in range(2):
                            nc.tensor.matmul(
                                pss[mt_i][half][:],
                                x_slice(sc, ko, mt_i),
                                wt[ko][half][:],
                                start=ko == 0,
                                stop=last,
                            )
                        if last:
                            for half in range(2):
                                evict_half(pss[mt_i][half], ots[mt_i], half)
                                nc.gpsimd.dma_start(
                                    y_v[:, mt, half * 512 : (half + 1) * 512],
                                    ots[mt_i][:, half * 512 : (half + 1) * 512],
                                )

            # Last super-chunk: mt-outer, so evictions and stores spread across
            # its whole span instead of piling up after the final matmul; the
            # very last m-tile runs its two output halves back to back so
            # half 0's eviction/store hides under half 1's matmuls.
            sc = n_sc - 1
            for mt_i in range(MPC):
                mt = sc * MPC + mt_i
                ot = opool.tile([P, O], bf16, tag="ot", name=f"otf{mt_i}")
                final = mt_i == MPC - 1
                if not final:
                    ph = [
                        psum.tile([P, 512], mybir.dt.float32, tag="ps", name=f"pl{h}")
                        for h in range(2)
                    ]
                    for ko in range(KO):
                        for half in range(2):
                            nc.tensor.matmul(
                                ph[half][:],
                                x_slice(sc, ko, mt_i),
                                wt[ko][half][:],
                                start=ko == 0,
                                stop=ko == KO - 1,
                            )
                    for half in range(2):
                        evict_half(ph[half], ot, half)
                        nc.gpsimd.dma_start(
                            y_v[:, mt, half * 512 : (half + 1) * 512],
                            ot[:, half * 512 : (half + 1) * 512],
                        )
                else:
                    for half in range(2):
                        ps = psum.tile([P, 512], mybir.dt.float32, tag="ps", name="pf")
                        for ko in range(KO):
                            nc.tensor.matmul(
                                ps[:],
                                x_slice(sc, ko, mt_i),
                                wt[ko][half][:],
                                start=ko == 0,
                                stop=ko == KO - 1,
                            )
                        evict_half(ps, ot, half)
                        nc.sync.dma_start(
                            y_v[:, mt, half * 512 : (half + 1) * 512],
                            ot[:, half * 512 : (half + 1) * 512],
                        )

    nc.compile()
    return nc


def _get_nc():
    global _NC_CACHE
    if _NC_CACHE is None:
        _NC_CACHE = _build_nc()
    return _NC_CACHE


def kernel(x, W, b, A, B):
    global LAST_RESULT
    x = np.ascontiguousarray(np.asarray(x, dtype=np.float32))
    W = np.asarray(W, dtype=np.float32)
    b = np.asarray(b, dtype=np.float32)
    A = np.asarray(A, dtype=np.float32)
    B = np.asarray(B, dtype=np.float32)
    assert x.shape == (4, 4096, D) and W.shape == (O, D)
    assert b.shape == (O,) and A.shape[1] == D and B.shape[0] == O

    # Fold the LoRA update into the weight: x@W^T + s*(x@A^T)@B^T = x@(W + s*B@A)^T
    Weff = (
        W.astype(np.float64) + SCALING * (B.astype(np.float64) @ A.astype(np.float64))
    ).astype(np.float32)
    WeffT = np.ascontiguousarray(Weff.T).astype(ml_dtypes.bfloat16)  # [D, O]
    bias_rep = np.ascontiguousarray(np.broadcast_to(b[None, :], (P, O)))

    xr = x.reshape(M_TOTAL, D)
    in_maps = []
    for c in range(N_CORES):
        xTc = np.ascontiguousarray(xr[c * M : (c + 1) * M].T).astype(
            ml_dtypes.bfloat16
        )  # [D, M]
        in_maps.append({"xT": xTc, "wT": WeffT, "bias": bias_rep})

    nc = _get_nc()
    res = run_bass_kernel_spmd(
        nc, in_maps, core_ids=list(range(N_CORES)), trace=TRACE
    )
    LAST_RESULT = res

    out = np.concatenate(
        [res.results[c]["y"].astype(np.float32) for c in range(N_CORES)], axis=0
    )
    return out.reshape(x.shape[0], x.shape[1], O)


# revision 7
# speedup vs baseline: 1.0787x; 1.0235x over previous
"""LoRA linear layer on 8 Trainium2 NeuronCores.

Computes y = x @ W^T + b + 2.0 * (x @ A^T) @ B^T for
x:[4,4096,1024], W:[1024,1024], b:[1024], A:[16,1024], B:[1024,16].

Host side folds the LoRA update into the weight (W_eff = W + 2*B@A, an exact
algebraic identity), so the device kernel is a single GEMM + bias. Sharding is
data-parallel over the 16384 tokens: each of the 8 cores computes a
[2048, 1024] output slice with replicated weights.

Device kernel (per core): y_c[m,o] = sum_d xT_c[d,m] * WeffT[d,o] + b[o]
  - xT_c  [1024, 2048] bf16 (host-transposed + cast; contraction dim d on
    SBUF partitions for both matmul operands)
  - WeffT [1024, 1024] bf16, fully resident in SBUF as 16 [128,512] half-tiles
  - bf16 matmuls at N=512 (1 col/cycle, FWL weight loads), fp32 PSUM
  - bias (fp32, broadcast to 128 partitions on host) fused into the DVE
    PSUM->SBUF eviction, output cast to bf16 there; host upcasts to fp32
"""

import numpy as np
import ml_dtypes

import concourse.mybir as mybir
import concourse.tile as tile
from concourse import bacc
from concourse.bass_utils import run_bass_kernel_spmd

N_CORES = 8
P = 128
D = 1024  # in_features (contraction)
O = 1024  # out_features
M_TOTAL = 4 * 4096  # tokens
M = M_TOTAL // N_CORES  # tokens per core
KO = D // P  # k-subtiles
SC = 512  # m super-chunk (DMA granularity)
SCALING = 2.0
N_WARM = 20  # cold-rate N=128 warmup matmuls bridging the first DMA latency

# Set by test harnesses to capture profiling info; harmless otherwise.
TRACE = False
LAST_RESULT = None

_NC_CACHE = None


def _build_nc():
    f32 = mybir.dt.float32
    bf16 = mybir.dt.bfloat16

    nc = bacc.Bacc("TRN2", debug=False)
    xT = nc.dram_tensor("xT", [D, M], bf16, kind="ExternalInput")
    wT = nc.dram_tensor("wT", [D, O], bf16, kind="ExternalInput")
    bias = nc.dram_tensor("bias", [P, O], f32, kind="ExternalInput")
    y = nc.dram_tensor("y", [M, O], bf16, kind="ExternalOutput")

    xT_v = xT[:].rearrange("(ko p) m -> p ko m", p=P)  # [128, 8, 2048]
    wT_v = wT[:].rearrange("(ko p) o -> p ko o", p=P)  # [128, 8, 1024]
    y_v = y[:].rearrange("(mt p) o -> p mt o", p=P)  # [128, 16, 1024]

    n_sc = M // SC
    with tile.TileContext(nc) as tc:
        with (
            tc.tile_pool(name="wpool", bufs=1) as wpool,
            tc.tile_pool(name="bpool", bufs=1) as bpool,
            tc.tile_pool(name="xpool", bufs=16) as xpool,
            tc.tile_pool(name="opool", bufs=6) as opool,
            tc.tile_pool(name="psum", bufs=8, space="PSUM") as psum,
        ):
            xts = {}

            # Loads alternate between the two HWDGE rings (SP via nc.sync,
            # ACT via nc.scalar): halves per-ring issue serialization and
            # gets the first granules to SBUF sooner.
            ring = [0]

            def load_dma(dst, src):
                eng = nc.sync if ring[0] % 2 == 0 else nc.scalar
                ring[0] += 1
                eng.dma_start(dst, src)

            def load_x(sc):
                for ko in range(KO):
                    t = xpool.tile([P, SC], bf16, tag="xt", name=f"x{sc}_{ko}")
                    load_dma(t[:], xT_v[:, ko, sc * SC : (sc + 1) * SC])
                    xts[(sc, ko)] = t

            # W half-tiles [128, 512]: the first matmul group gates on 128 KiB
            # instead of 256 KiB.
            wt = [[None, None] for _ in range(KO)]

            def load_w(ko):
                for h in range(2):
                    t = wpool.tile([P, 512], bf16, tag=f"w{ko}_{h}")
                    load_dma(t[:], wT_v[:, ko, h * 512 : (h + 1) * 512])
                    wt[ko][h] = t

            # Warmup: short N=128 matmuls on a zeroed tile keep the PE busy
            # from ~body start so the HAM clock-gate window (3.4 us of
            # sustained activity) elapses while the first x/W slices stream
            # in; real matmuls then take over and finish the warm-up.
            zt = wpool.tile([P, P], bf16, tag="warm")
            nc.gpsimd.memset(zt[:], 0.0)
            wps = psum.tile([P, 512], mybir.dt.float32, tag="ps", name="wps")
            for _ in range(N_WARM):
                nc.tensor.matmul(wps[:, :P], zt[:], zt[:], start=True, stop=True)

            # sc0's x interleaved with W slices so the first real matmul only
            # waits on ~0.25 MiB; bias right behind (it gates all evictions).
            for ko in range(KO):
                t = xpool.tile([P, SC], bf16, tag="xt", name=f"x0_{ko}")
                load_dma(t[:], xT_v[:, ko, 0:SC])
                xts[(0, ko)] = t
                load_w(ko)
            bt = bpool.tile([P, O], f32)
            load_dma(bt[:], bias[:])

            def x_slice(sc, ko, mt_i):
                return xts[(sc, ko)][:, mt_i * P : (mt_i + 1) * P]

            def evict_half(ps, ot, half):
                nc.vector.tensor_tensor(
                    ot[:, half * 512 : (half + 1) * 512],
                    ps[:],
                    bt[:, half * 512 : (half + 1) * 512],
                    mybir.AluOpType.add,
                )

            MPC = SC // P  # m-tiles per super-chunk

            # Every super-chunk runs ko-outer: all four m-tiles accumulate
            # simultaneously across the 8 single-bank PSUM groups, so each W/x
            # slice is consumed as it lands during the ramp and the PE never
            # sits behind one large dependency. Evictions + stores are inlined
            # right behind each group's stop so PSUM slots recycle smoothly
            # into the next super-chunk.
            for sc in range(n_sc - 1):
                if sc + 1 < n_sc:
                    load_x(sc + 1)
                pss = [
                    [
                        psum.tile(
                            [P, 512], mybir.dt.float32, tag="ps", name=f"ps{sc}_{i}_{h}"
                        )
                        for h in range(2)
                    ]
                    for i in range(MPC)
                ]
                ots = [
                    opool.tile([P, O], bf16, tag="ot", name=f"ot{sc}_{i}")
                    for i in range(MPC)
                ]
                for ko in range(KO):
                    last = ko == KO - 1
                    for mt_i in range(MPC):
                        mt = sc * MPC + mt_i
                        for half in range(2):
                            nc.tensor.matmul(
                                pss[mt_i][half][:],
                                x_slice(sc, ko, mt_i),
                                wt[ko][half][:],
                                start=ko == 0,
                                stop=last,
                            )
                        if last:
                            for half in range(2):
                                evict_half(pss[mt_i][half], ots[mt_i], half)
                                nc.gpsimd.dma_start(
                                    y_v[:, mt, half * 512 : (half + 1) * 512],
                                    ots[mt_i][:, half * 512 : (half + 1) * 512],
                                )

            # Last super-chunk: mt-outer, so evictions and stores spread across
            # its whole span instead of piling up after the final matmul; the
            # very last m-tile runs its two output halves back to back so
            # half 0's eviction/store hides under half 1's matmuls.
            sc = n_sc - 1
            for mt_i in range(MPC):
                mt = sc * MPC + mt_i
                ot = opool.tile([P, O], bf16, tag="ot", name=f"otf{mt_i}")
                final = mt_i == MPC - 1
                if not final:
                    ph = [
                        psum.tile([P, 512], mybir.dt.float32, tag="ps", name=f"pl{h}")
                        for h in range(2)
                    ]
                    for ko in range(KO):
                        for half in range(2):
                            nc.tensor.matmul(
                                ph[half][:],
                                x_slice(sc, ko, mt_i),
                                wt[ko][half][:],
                                start=ko == 0,
                                stop=ko == KO - 1,
                            )
                    for half in range(2):
                        evict_half(ph[half], ot, half)
                        nc.gpsimd.dma_start(
                            y_v[:, mt, half * 512 : (half + 1) * 512],
                            ot[:, half * 512 : (half + 1) * 512],
                        )
                else:
                    # Last m-tile: half 0 as one N=512 group, half 1 as two
                    # N=256 quarter groups (same PE cycles) so the final
                    # eviction+store handles only 64 KiB — its HBM write
                    # receipt is the very last thing the teardown waits on.
                    ps = psum.tile([P, 512], mybir.dt.float32, tag="ps", name="pf0")
                    for ko in range(KO):
                        nc.tensor.matmul(
                            ps[:],
                            x_slice(sc, ko, mt_i),
                            wt[ko][0][:],
                            start=ko == 0,
                            stop=ko == KO - 1,
                        )
                    evict_half(ps, ot, 0)
                    nc.sync.dma_start(y_v[:, mt, 0:512], ot[:, 0:512])
                    for q in range(2):
                        qs = psum.tile(
                            [P, 256], mybir.dt.float32, tag="ps", name=f"pfq{q}"
                        )
                        for ko in range(KO):
                            nc.tensor.matmul(
                                qs[:],
                                x_slice(sc, ko, mt_i),
                                wt[ko][1][:, q * 256 : (q + 1) * 256],
                                start=ko == 0,
                                stop=ko == KO - 1,
                            )
                        lo = 512 + q * 256
                        nc.vector.tensor_tensor(
                            ot[:, lo : lo + 256],
                            qs[:],
                            bt[:, lo : lo + 256],
                            mybir.AluOpType.add,
                        )
                        nc.sync.dma_start(
                            y_v[:, mt, lo : lo + 256], ot[:, lo : lo + 256]
                        )

    nc.compile()
    return nc


def _get_nc():
    global _NC_CACHE
    if _NC_CACHE is None:
        _NC_CACHE = _build_nc()
    return _NC_CACHE


def kernel(x, W, b, A, B):
    global LAST_RESULT
    x = np.ascontiguousarray(np.asarray(x, dtype=np.float32))
    W = np.asarray(W, dtype=np.float32)
    b = np.asarray(b, dtype=np.float32)
    A = np.asarray(A, dtype=np.float32)
    B = np.asarray(B, dtype=np.float32)
    assert x.shape == (4, 4096, D) and W.shape == (O, D)
    assert b.shape == (O,) and A.shape[1] == D and B.shape[0] == O

    # Fold the LoRA update into the weight: x@W^T + s*(x@A^T)@B^T = x@(W + s*B@A)^T
    Weff = (
        W.astype(np.float64) + SCALING * (B.astype(np.float64) @ A.astype(np.float64))
    ).astype(np.float32)
    WeffT = np.ascontiguousarray(Weff.T).astype(ml_dtypes.bfloat16)  # [D, O]
    bias_rep = np.ascontiguousarray(np.broadcast_to(b[None, :], (P, O)))

    xr = x.reshape(M_TOTAL, D)
    in_maps = []
    for c in range(N_CORES):
        xTc = np.ascontiguousarray(xr[c * M : (c + 1) * M].T).astype(
            ml_dtypes.bfloat16
        )  # [D, M]
        in_maps.append({"xT": xTc, "wT": WeffT, "bias": bias_rep})

    nc = _get_nc()
    res = run_bass_kernel_spmd(
        nc, in_maps, core_ids=list(range(N_CORES)), trace=TRACE
    )
    LAST_RESULT = res

    out = np.concatenate(
        [res.results[c]["y"].astype(np.float32) for c in range(N_CORES)], axis=0
    )
    return out.reshape(x.shape[0], x.shape[1], O)
